# revision 19
# baseline (speedup 1.0000x reference)
"""MetaLoss (segment-reduce BCE) kernel for 8 Trainium2 NeuronCores.

Computation (see reference):
    log1m   = log_sigmoid(-logits) = -softplus(logits)              [B, L]
    log_meta[b,g] = sum_{l: gid[l]==g} log1m[b,l]                   [B, G]
    meta_y  = (segment_sum(true_y) > 0)                             [B, G]
    bce     = -mean(meta_y * max(log_meta, -100)
                    + (1-meta_y) * max(log1p(-exp(log_meta)), -100))

Sharding: data-parallel over batch rows, 1024 rows/core. group_ids is
arange(L) % G, so group g's members are columns {g, g+G, ..., g+7G}; the
8-way segment sum becomes a sum over 8 contiguous 512-wide chunks.

Device strategy per core (memory-bound; 32 MiB of input / core):
  - blocks of 128 batch rows in natural [row, label] layout (contiguous
    2 MiB DMAs).
  - log_sigmoid(-x) = Ln(Sigmoid(-x)) on ScalarE over the whole
    [128, 4096] block (two instrs; Softplus LUT is not supported by the
    toolchain everywhere, Sigmoid/Ln are).
  - the 8-chunk segment sum runs on TensorE: 8 accumulating identity-
    weight matmuls, one per 512-wide chunk, into a PSUM bank —
    psum[r, g] += chunk_k[r, g], giving log_meta directly. The log-
    sigmoid side runs in fp32 (exact); the true_y side is loaded via a
    casting gpsimd DMA as bf16 (exact for 0/1 counts) at full PE rate.
  - BCE tail on [128, 512] tiles: Exp/Ln on ScalarE, clamp/select/reduce
    on VectorE (copy_predicated with the y-count PSUM tile as mask).
  - per-(row, block) partial sums [128, 8] per core are summed on host
    (the "final scalar all-reduce" of the sharding hint).
"""

import numpy as np

import concourse.bass as bass
import concourse.tile as tile
from concourse import bacc, mybir
from concourse.bass_utils import run_bass_kernel_spmd

B, L, G = 8192, 4096, 512
META_PARAM = 1.0
LOG_CLAMP = -100.0

NCORES = 8
ROWS = B // NCORES          # 1024 rows per core
KCH = L // G                # 8 labels (chunks) per group
BLK = 128                   # batch rows per block
NBLK = ROWS // BLK          # 8 blocks per core

_F32 = mybir.dt.float32
_BF16 = mybir.dt.bfloat16
_AFT = mybir.ActivationFunctionType
_ALU = mybir.AluOpType

_NC = None                  # cached Bass program (compile once per process)
LAST_RESULT = None          # BassKernelResults of the most recent device run
TRACE = False               # set True (e.g. from test.py) to profile
TRACE_DIR = None            # optional tmpdir for NTFF/perfetto artifacts


def _build_bass(repeat: int = 1):
    """repeat>1 re-runs the whole per-core pipeline on the same inputs —
    used only for wall-clock slope benchmarking (bench.py)."""
    nc = bacc.Bacc(
        "TRN2",
        target_bir_lowering=False,
        debug=False,
        num_devices=NCORES,
    )
    logits = nc.dram_tensor("logits", [ROWS, L], _F32, kind="ExternalInput").ap()
    true_y = nc.dram_tensor("true_y", [ROWS, L], _F32, kind="ExternalInput").ap()
    ident = nc.dram_tensor("ident", [128, 128], _F32, kind="ExternalInput").ap()
    out_acc = nc.dram_tensor("acc", [128, NBLK], _F32, kind="ExternalOutput").ap()

    lg = logits.rearrange("(nb p) l -> nb p l", p=BLK)
    yg = true_y.rearrange("(nb p) l -> nb p l", p=BLK)

    with tile.TileContext(nc) as tc:
        with (
            tc.tile_pool(name="big", bufs=3) as big_pool,
            tc.tile_pool(name="sp", bufs=2) as sp_pool,
            tc.tile_pool(name="small", bufs=2) as small_pool,
            tc.tile_pool(name="const", bufs=1) as const_pool,
            tc.tile_pool(name="psum", bufs=2, space=bass.MemorySpace.PSUM) as psum_pool,
        ):
            id_f = const_pool.tile([128, 128], _F32)
            nc.sync.dma_start(id_f[:], ident)
            id_h = const_pool.tile([128, 128], _BF16)
            nc.gpsimd.dma_start(id_h[:], ident)  # casting DMA f32 -> bf16
            acc_t = const_pool.tile([128, NBLK], _F32)

            for b in [b for _ in range(repeat) for b in range(NBLK)]:
                lt = big_pool.tile([128, L], _F32, tag="lt")
                nc.sync.dma_start(lt[:], lg[b])
                yt = big_pool.tile([128, L], _BF16, tag="yt")
                nc.gpsimd.dma_start(yt[:], yg[b])  # casting DMA f32 -> bf16

                # ls = log_sigmoid(-x) = Ln(Sigmoid(-x))
                sg_t = sp_pool.tile([128, L], _F32, tag="sg")
                nc.scalar.activation(sg_t[:], lt[:], _AFT.Sigmoid, scale=-1.0)
                ls_t = sp_pool.tile([128, L], _F32, tag="ls")
                nc.scalar.activation(ls_t[:], sg_t[:], _AFT.Ln)

                psum_lm = psum_pool.tile([128, G], _F32, tag="plm")
                psum_y = psum_pool.tile([128, G], _F32, tag="py")
                for k in range(KCH):
                    nc.tensor.matmul(
                        psum_lm[:],
                        id_f[:],
                        ls_t[:, k * G : (k + 1) * G],
                        start=(k == 0),
                        stop=(k == KCH - 1),
                    )
                for k in range(KCH):
                    nc.tensor.matmul(
                        psum_y[:],
                        id_h[:],
                        yt[:, k * G : (k + 1) * G],
                        start=(k == 0),
                        stop=(k == KCH - 1),
                    )

                # p = exp(log_meta);  lnq = ln(1 - p)
                p_t = small_pool.tile([128, G], _F32, tag="p")
                nc.scalar.activation(p_t[:], psum_lm[:], _AFT.Exp)
                lnq_t = small_pool.tile([128, G], _F32, tag="lnq")
                nc.scalar.activation(
                    lnq_t[:], p_t[:], _AFT.Ln, bias=1.0, scale=-1.0
                )

                # log_p = max(log_meta, -100)
                logp_t = small_pool.tile([128, G], _F32, tag="logp")
                nc.vector.tensor_scalar_max(logp_t[:], psum_lm[:], LOG_CLAMP)
                # term = max(lnq, -100), overwritten with log_p where ycount>0
                term_t = small_pool.tile([128, G], _F32, tag="term")
                nc.vector.tensor_scalar_max(term_t[:], lnq_t[:], LOG_CLAMP)
                nc.vector.copy_predicated(
                    term_t[:], psum_y[:].bitcast(mybir.dt.uint32), logp_t[:]
                )
                nc.vector.tensor_reduce(
                    acc_t[:, b : b + 1], term_t[:], mybir.AxisListType.X, _ALU.add
                )

            nc.sync.dma_start(out_acc, acc_t[:])

    nc.compile()
    return nc


def _get_nc():
    global _NC
    if _NC is None:
        _NC = _build_bass()
    return _NC


def _ident_np():
    return np.eye(128, dtype=np.float32)


def _numpy_reference(logits, true_y, group_ids):
    """Generic host fallback (never hit for the spec'd inputs)."""
    log1m = -np.logaddexp(0.0, logits.astype(np.float64))
    gid = np.asarray(group_ids).astype(np.int64)
    log_meta = np.zeros((logits.shape[0], G), dtype=np.float64)
    ysum = np.zeros((logits.shape[0], G), dtype=np.float64)
    for g in range(G):
        cols = np.nonzero(gid == g)[0]
        log_meta[:, g] = log1m[:, cols].sum(axis=1)
        ysum[:, g] = true_y[:, cols].sum(axis=1)
    meta_y = (ysum > 0).astype(np.float64)
    log_p = np.maximum(log_meta, LOG_CLAMP)
    log_1mp = np.maximum(np.log1p(-np.exp(log_meta)), LOG_CLAMP)
    bce = -np.mean(meta_y * log_p + (1.0 - meta_y) * log_1mp)
    return np.asarray(bce * META_PARAM, dtype=np.float32)


def kernel(logits, true_y, group_ids):
    global LAST_RESULT
    logits = np.ascontiguousarray(np.asarray(logits), dtype=np.float32)
    true_y = np.ascontiguousarray(np.asarray(true_y), dtype=np.float32)
    gid = np.asarray(group_ids)

    if not np.array_equal(gid.astype(np.int64), np.arange(L, dtype=np.int64) % G):
        counts = np.bincount(gid.astype(np.int64), minlength=G)
        if gid.shape == (L,) and counts.min() == KCH and counts.max() == KCH:
            # balanced groups: permute columns so column k*G+g is group
            # g's k-th member, which restores the strided layout.
            order = np.argsort(gid, kind="stable").reshape(G, KCH).T.reshape(-1)
            logits = np.ascontiguousarray(logits[:, order])
            true_y = np.ascontiguousarray(true_y[:, order])
        else:
            return _numpy_reference(logits, true_y, gid)

    nc = _get_nc()
    ident = _ident_np()
    sl = logits.reshape(NCORES, ROWS, L)
    sy = true_y.reshape(NCORES, ROWS, L)
    in_maps = [
        {"logits": sl[c], "true_y": sy[c], "ident": ident} for c in range(NCORES)
    ]
    res = run_bass_kernel_spmd(
        nc, in_maps, list(range(NCORES)), trace=TRACE, tmpdir=TRACE_DIR
    )
    LAST_RESULT = res
    total = 0.0
    for r in res.results:
        total += float(r["acc"].astype(np.float64).sum())
    out = -(total / (B * G)) * META_PARAM
    return np.asarray(out, dtype=np.float32)
